# revision 1
# baseline (speedup 1.0000x reference)
"""Trainium2 Bass kernel for a dense transformer encoder layer.

Model (faithful to the oracle):
  q,k,v = x@wq+bq, x@wk+bk, x@wv+bv          (12 heads, dk=64, DIM=768)
  scores = q@k^T / sqrt(768)  (note: sqrt(dim_model), not sqrt(dk))
  scores[mask==0] = 1e-11  (NOT -inf; masked keys still contribute ~1/Z)
  attn = softmax(scores); z = attn@v; o = z@wo+bo
  l1 = x + LN(o);  ffn = relu(l1@w1+b1)@w2+b2;  out = l1 + LN(ffn)

Sharding: 4096 tokens (B=2,S=2048) split 8 ways -> 512 tokens/core.
Cores 0-3 own batch 0, cores 4-7 batch 1. K/V are computed for the
core's whole batch (redundantly within each 4-core group) so attention
needs no collectives.

Softmax trick: scores are built k-major (scoresT [kpos, q]) so the
mask (per-k) is a per-partition scalar; exp(mask_p/sqrt(768) * s) on
the scalar engine applies scale+mask+exp in a single pass (masked rows
give exp(0)=1.0 == fp32(exp(1e-11))). The denominator comes from a
ones column appended to V (attn@v with M=65); normalization happens
after attn@v via a rank-1 matmul broadcast of 1/sum.
"""

import math
import os
import sys

import numpy as np

for _p in ("/opt/trn_rl_repo", os.path.expanduser("~/.axon_site/_ro/trn_rl_repo")):
    if os.path.isdir(_p) and _p not in sys.path:
        sys.path.insert(0, _p)

import ml_dtypes  # noqa: E402

BF16 = ml_dtypes.bfloat16

DIM = 768
HEADS = 12
DK = 64
HID = 4 * DIM  # 3072
B, S = 2, 2048
N_CORES = 8
BLK = 512            # tokens per core
NBLK = S // BLK      # 4 blocks per batch
EPS = 1e-5
ISCALE = 1.0 / math.sqrt(DIM)

_CACHE: dict = {}
MAX_PHASE = int(os.environ.get("BASS_KERNEL_PHASES", "5"))
USE_AG = os.environ.get("BASS_KERNEL_AG", "1") == "1"


def _build_program():
    import concourse.bass as bass
    import concourse.mybir as mybir
    import concourse.tile as tile
    from concourse import bacc
    from concourse.masks import make_identity

    f32 = mybir.dt.float32
    bf16 = mybir.dt.bfloat16
    AF = mybir.ActivationFunctionType
    ALU = mybir.AluOpType
    AX = mybir.AxisListType

    nc = bacc.Bacc()

    # ---- per-core DRAM I/O ----
    if not USE_AG:
        d_xT = nc.dram_tensor("xT", [DIM, S], bf16, kind="ExternalInput")
    d_xTb = nc.dram_tensor("xTb", [DIM, BLK], bf16, kind="ExternalInput")
    d_xb = nc.dram_tensor("xb", [BLK, DIM], f32, kind="ExternalInput")
    d_msc = nc.dram_tensor("msc", [S], f32, kind="ExternalInput")
    d_wq = nc.dram_tensor("wq", [DIM, DIM], bf16, kind="ExternalInput")
    d_wk = nc.dram_tensor("wk", [DIM, DIM], bf16, kind="ExternalInput")
    d_wv = nc.dram_tensor("wv", [DIM, DIM], bf16, kind="ExternalInput")
    d_wo = nc.dram_tensor("wo", [DIM, DIM], bf16, kind="ExternalInput")
    d_w1 = nc.dram_tensor("w1", [DIM, HID], bf16, kind="ExternalInput")
    d_w2 = nc.dram_tensor("w2", [HID, DIM], bf16, kind="ExternalInput")
    d_bq = nc.dram_tensor("bq", [DIM], f32, kind="ExternalInput")
    d_bk = nc.dram_tensor("bk", [DIM], f32, kind="ExternalInput")
    d_bv = nc.dram_tensor("bv", [DIM], f32, kind="ExternalInput")
    d_bo = nc.dram_tensor("bo", [DIM], f32, kind="ExternalInput")
    d_b1 = nc.dram_tensor("b1", [HID], f32, kind="ExternalInput")
    d_b2 = nc.dram_tensor("b2", [DIM], f32, kind="ExternalInput")
    d_g1 = nc.dram_tensor("g1", [DIM], f32, kind="ExternalInput")
    d_bb1 = nc.dram_tensor("bb1", [DIM], f32, kind="ExternalInput")
    d_g2 = nc.dram_tensor("g2", [DIM], f32, kind="ExternalInput")
    d_bb2 = nc.dram_tensor("bb2", [DIM], f32, kind="ExternalInput")
    d_out = nc.dram_tensor("out", [BLK, DIM], f32, kind="ExternalOutput")
    if USE_AG:
        d_kb = nc.dram_tensor("k_bounce", [DIM, BLK], bf16)
        d_ks = nc.dram_tensor("k_shared", [NBLK * DIM, BLK], bf16)
        d_vb = nc.dram_tensor("v_bounce", [BLK, HEADS * (DK + 1)], bf16)
        d_vs = nc.dram_tensor("v_shared", [S, HEADS * (DK + 1)], bf16)
        RG = [[0, 1, 2, 3], [4, 5, 6, 7]]

    FT = DIM // 128   # 6 feature tiles
    TT = BLK // 128   # 4 token tiles per core block
    ST = S // 128     # 16 token tiles per batch
    HT = HID // 128   # 24 hidden tiles

    def bcast_ap(handle, n=128):
        ap = handle[:]
        return bass.AP(tensor=ap.tensor, offset=ap.offset, ap=[[0, n]] + list(ap.ap))

    with tile.TileContext(nc) as tc:
        with (
            tc.tile_pool(name="const", bufs=1) as const,
            tc.tile_pool(name="bigres", bufs=1) as big,
        ):
            # ---------- constants ----------
            sb_msc = const.tile([128, ST], f32)
            nc.sync.dma_start(out=sb_msc, in_=d_msc[:].rearrange("(t p) -> p t", p=128))
            sb_bq = const.tile([128, FT], f32)
            nc.sync.dma_start(out=sb_bq, in_=d_bq[:].rearrange("(t p) -> p t", p=128))
            sb_bk = const.tile([128, FT], f32)
            nc.sync.dma_start(out=sb_bk, in_=d_bk[:].rearrange("(t p) -> p t", p=128))
            sb_b1 = const.tile([128, HT], f32)
            nc.sync.dma_start(out=sb_b1, in_=d_b1[:].rearrange("(t p) -> p t", p=128))
            bv_bc = const.tile([128, DIM], f32)
            nc.gpsimd.dma_start(out=bv_bc, in_=bcast_ap(d_bv))
            bo_bc = const.tile([128, DIM], f32)
            nc.gpsimd.dma_start(out=bo_bc, in_=bcast_ap(d_bo))
            b2_bc = const.tile([128, DIM], f32)
            nc.gpsimd.dma_start(out=b2_bc, in_=bcast_ap(d_b2))
            g1_bc = const.tile([128, DIM], f32)
            nc.gpsimd.dma_start(out=g1_bc, in_=bcast_ap(d_g1))
            bb1_bc = const.tile([128, DIM], f32)
            nc.gpsimd.dma_start(out=bb1_bc, in_=bcast_ap(d_bb1))
            g2_bc = const.tile([128, DIM], f32)
            nc.gpsimd.dma_start(out=g2_bc, in_=bcast_ap(d_g2))
            bb2_bc = const.tile([128, DIM], f32)
            nc.gpsimd.dma_start(out=bb2_bc, in_=bcast_ap(d_bb2))
            ident = const.tile([128, 128], f32)
            make_identity(nc, ident[:])
            ones64 = const.tile([1, 64], f32)
            nc.vector.memset(ones64, 1.0)
            eps_t = const.tile([128, 1], f32)
            nc.vector.memset(eps_t, EPS)

            # ---------- persistent activations ----------
            sb_xblk = big.tile([128, TT, DIM], f32)  # residual x
            sb_l1 = big.tile([128, TT, DIM], f32)

            nc.sync.dma_start(
                out=sb_xblk, in_=d_xb[:].rearrange("(t p) d -> p t d", p=128)
            )

            # attention-scoped residents (freed before the FFN phases)
            attn_res_cm = tc.tile_pool(name="attn_res", bufs=1)
            attn_res = attn_res_cm.__enter__()
            sb_K = attn_res.tile([128, FT, NBLK, BLK], bf16)  # K^T, feat-major
            sb_Q = attn_res.tile([128, FT, BLK], bf16)  # Q^T, feat-major
            sb_V = attn_res.tile([128, ST, HEADS, DK + 1], bf16)  # V + ones col
            sb_zT = attn_res.tile([128, FT, BLK], bf16)  # z^T normalized

            # ============ Phase 1: QKV projections ============
            with (
                tc.tile_pool(name="xw", bufs=1) as xw,
                tc.tile_pool(name="ps1", bufs=4, space="PSUM") as ps1,
                tc.tile_pool(name="ps1v", bufs=4, space="PSUM") as ps1v,
            ):
                if not USE_AG:
                    sb_xT = xw.tile([128, FT, S], bf16)
                    nc.sync.dma_start(
                        out=sb_xT, in_=d_xT[:].rearrange("(t p) n -> p t n", p=128)
                    )
                sb_xTb = xw.tile([128, FT, BLK], bf16)
                nc.sync.dma_start(
                    out=sb_xTb, in_=d_xTb[:].rearrange("(t p) n -> p t n", p=128)
                )
                w_q = xw.tile([128, FT, DIM], bf16)
                nc.sync.dma_start(
                    out=w_q, in_=d_wq[:].rearrange("(t p) o -> p t o", p=128)
                )
                w_k = xw.tile([128, FT, DIM], bf16)
                nc.sync.dma_start(
                    out=w_k, in_=d_wk[:].rearrange("(t p) o -> p t o", p=128)
                )
                w_v = xw.tile([128, FT, DIM], bf16)
                nc.sync.dma_start(
                    out=w_v, in_=d_wv[:].rearrange("(t p) o -> p t o", p=128)
                )

                if USE_AG:
                    # K^T feat-major for the own block only -> bounce -> AG
                    kstage = xw.tile([128, FT, BLK], bf16, tag="kstage")
                    for ft in range(FT):
                        ps = ps1.tile([128, 512], f32, tag="p")
                        for kt in range(FT):
                            nc.tensor.matmul(
                                ps,
                                w_k[:, kt, ft * 128 : (ft + 1) * 128],
                                sb_xTb[:, kt, :],
                                start=(kt == 0),
                                stop=(kt == FT - 1),
                            )
                        nc.vector.tensor_scalar_add(
                            kstage[:, ft, :], ps, sb_bk[:, ft : ft + 1]
                        )
                    nc.sync.dma_start(
                        out=d_kb[:].rearrange("(t p) n -> p t n", p=128), in_=kstage
                    )
                else:
                    # K^T feat-major over the whole batch (replicated)
                    for ft in range(FT):
                        for nt in range(S // 512):
                            ps = ps1.tile([128, 512], f32, tag="p")
                            for kt in range(FT):
                                nc.tensor.matmul(
                                    ps,
                                    w_k[:, kt, ft * 128 : (ft + 1) * 128],
                                    sb_xT[:, kt, nt * 512 : (nt + 1) * 512],
                                    start=(kt == 0),
                                    stop=(kt == FT - 1),
                                )
                            nc.vector.tensor_scalar_add(
                                sb_K[:, ft, nt, :], ps, sb_bk[:, ft : ft + 1]
                            )
                # Q^T feat-major for the core's block
                for ft in range(FT):
                    ps = ps1.tile([128, 512], f32, tag="p")
                    for kt in range(FT):
                        nc.tensor.matmul(
                            ps,
                            w_q[:, kt, ft * 128 : (ft + 1) * 128],
                            sb_xTb[:, kt, :],
                            start=(kt == 0),
                            stop=(kt == FT - 1),
                        )
                    nc.vector.tensor_scalar_add(
                        sb_Q[:, ft, :], ps, sb_bq[:, ft : ft + 1]
                    )
                if USE_AG:
                    # V tok-major for the own block -> bounce -> AG
                    vstage = xw.tile([128, TT, HEADS, DK + 1], bf16, tag="vstage")
                    nc.vector.memset(vstage[:, :, :, DK : DK + 1], 1.0)
                    for tt in range(TT):
                        for nh in range(2):
                            ps = ps1v.tile([128, 384], f32, tag="vp")
                            for kt in range(FT):
                                nc.tensor.matmul(
                                    ps,
                                    sb_xTb[:, kt, tt * 128 : (tt + 1) * 128],
                                    w_v[:, kt, nh * 384 : (nh + 1) * 384],
                                    start=(kt == 0),
                                    stop=(kt == FT - 1),
                                )
                            nc.vector.scalar_tensor_tensor(
                                out=vstage[:, tt, nh * 6 : (nh + 1) * 6, 0:DK],
                                in0=ps[:].rearrange("p (h d) -> p h d", d=DK),
                                scalar=1.0,
                                in1=bv_bc[:, nh * 384 : (nh + 1) * 384].rearrange(
                                    "p (h d) -> p h d", d=DK
                                ),
                                op0=ALU.mult,
                                op1=ALU.add,
                            )
                    nc.sync.dma_start(
                        out=d_vb[:].rearrange("(t p) (h d) -> p t h d", p=128, d=DK + 1),
                        in_=vstage,
                    )
                    # AllGather K and V across the 4-core batch group
                    nc.gpsimd.collective_compute(
                        "AllGather", ALU.bypass, replica_groups=RG,
                        ins=[d_kb[:]], outs=[d_ks[:]],
                    )
                    nc.gpsimd.collective_compute(
                        "AllGather", ALU.bypass, replica_groups=RG,
                        ins=[d_vb[:]], outs=[d_vs[:]],
                    )
                    for b in range(NBLK):
                        nc.sync.dma_start(
                            out=sb_K[:, :, b, :],
                            in_=d_ks[b * DIM : (b + 1) * DIM, :].rearrange(
                                "(t p) n -> p t n", p=128
                            ),
                        )
                    nc.sync.dma_start(
                        out=sb_V,
                        in_=d_vs[:].rearrange(
                            "(t p) (h d) -> p t h d", p=128, d=DK + 1
                        ),
                    )
                else:
                    # V tok-major over the whole batch, laid out [tok, head, dk+1]
                    nc.vector.memset(sb_V[:, :, :, DK : DK + 1], 1.0)
                    for nh in range(2):
                        for tt in range(ST):
                            ps = ps1v.tile([128, 384], f32, tag="vp")
                            for kt in range(FT):
                                nc.tensor.matmul(
                                    ps,
                                    sb_xT[:, kt, tt * 128 : (tt + 1) * 128],
                                    w_v[:, kt, nh * 384 : (nh + 1) * 384],
                                    start=(kt == 0),
                                    stop=(kt == FT - 1),
                                )
                            nc.vector.scalar_tensor_tensor(
                                out=sb_V[:, tt, nh * 6 : (nh + 1) * 6, 0:DK],
                                in0=ps[:].rearrange("p (h d) -> p h d", d=DK),
                                scalar=1.0,
                                in1=bv_bc[:, nh * 384 : (nh + 1) * 384].rearrange(
                                    "p (h d) -> p h d", d=DK
                                ),
                                op0=ALU.mult,
                                op1=ALU.add,
                            )

            if MAX_PHASE >= 2:
                # ============ Phase 2: attention ============
                with (
                    tc.tile_pool(name="expp", bufs=64) as expp,
                    tc.tile_pool(name="attsm", bufs=2) as attsm,
                    tc.tile_pool(name="ps_sc", bufs=4, space="PSUM") as ps_sc,
                    tc.tile_pool(name="ps_z", bufs=2, space="PSUM") as ps_z,
                    tc.tile_pool(name="ps_rb", bufs=1, space="PSUM") as ps_rb,
                ):
                    for hp in range(HEADS // 2):
                        ht = hp
                        # interleave the two heads of a pair kt-by-kt: their
                        # K=64 matmuls sit in disjoint PE row groups (0-63 /
                        # 64-127) so the hardware overlaps adjacent pairs.
                        ets = ([], [])
                        for kt2 in range(ST):
                            for half in (0, 1):
                                ho = half * 64
                                ps = ps_sc.tile([128, BLK], f32, tag="sc")
                                nc.tensor.matmul(
                                    ps,
                                    sb_K[ho : ho + 64, ht, kt2 // 4, (kt2 % 4) * 128 : (kt2 % 4) * 128 + 128],
                                    sb_Q[ho : ho + 64, ht, :],
                                    start=True,
                                    stop=True,
                                )
                                et = expp.tile([128, BLK], bf16, tag="exp")
                                nc.scalar.activation(
                                    et, ps, AF.Exp, scale=sb_msc[:, kt2 : kt2 + 1]
                                )
                                ets[half].append(et)
                        for half in (0, 1):
                            h = 2 * hp + half
                            ho = half * 64
                            zp = ps_z.tile([DK + 1, BLK], f32, tag="z")
                            for kt2 in range(ST):
                                nc.tensor.matmul(
                                    zp,
                                    sb_V[:, kt2, h, :],
                                    ets[half][kt2],
                                    start=(kt2 == 0),
                                    stop=(kt2 == ST - 1),
                                )
                            rsum = attsm.tile([1, BLK], f32, tag="rsum")
                            nc.vector.reciprocal(rsum, zp[DK : DK + 1, :])
                            rbp = ps_rb.tile([64, BLK], f32, tag="rb")
                            nc.tensor.matmul(
                                rbp, ones64[:], rsum, start=True, stop=True
                            )
                            rb = attsm.tile([64, BLK], f32, tag="rbs")
                            nc.vector.tensor_copy(rb, rbp)
                            nc.vector.tensor_mul(
                                sb_zT[ho : ho + 64, ht, :], zp[0:DK, :], rb
                            )

            if MAX_PHASE >= 3:
                # ============ Phase 3: O proj + LN1 (+residual) ============
                def layer_norm_to(out_ap, x_ap, g_bc_t, resid_ap, pool):
                    s = pool.tile([128, 1], f32, tag="ln_s")
                    nc.vector.tensor_reduce(s, x_ap, axis=AX.X, op=ALU.add)
                    mean = pool.tile([128, 1], f32, tag="ln_m")
                    nc.scalar.mul(mean, s, 1.0 / DIM)
                    xc = pool.tile([128, DIM], f32, tag="ln_xc")
                    nc.vector.tensor_scalar(xc, x_ap, mean, None, op0=ALU.subtract)
                    junk = pool.tile([128, DIM], f32, tag="ln_j")
                    var = pool.tile([128, 1], f32, tag="ln_v")
                    # (tensor_tensor_reduce crashes the device on this runtime;
                    # scalar_tensor_tensor with accum_out works)
                    nc.vector.scalar_tensor_tensor(
                        out=junk, in0=xc, scalar=1.0, in1=xc,
                        op0=ALU.mult, op1=ALU.mult, accum_out=var,
                    )
                    nc.vector.tensor_scalar_mul(var, var, 1.0 / DIM)
                    sd = pool.tile([128, 1], f32, tag="ln_sd")
                    nc.scalar.activation(sd, var, AF.Sqrt, bias=eps_t[:])
                    rstd = pool.tile([128, 1], f32, tag="ln_r")
                    nc.vector.reciprocal(rstd, sd)
                    t = pool.tile([128, DIM], f32, tag="ln_t")
                    nc.vector.tensor_scalar(t, xc, rstd, None, op0=ALU.mult)
                    tg = pool.tile([128, DIM], f32, tag="ln_tg")
                    nc.vector.tensor_mul(tg, t, g_bc_t)
                    nc.vector.tensor_add(out_ap, tg, resid_ap)

                with (
                    tc.tile_pool(name="wo_p", bufs=1) as wo_p,
                    tc.tile_pool(name="ln1p", bufs=2) as ln1p,
                    tc.tile_pool(name="ps_o", bufs=4, space="PSUM") as ps_o,
                ):
                    w_o = wo_p.tile([128, FT, DIM], bf16)
                    nc.sync.dma_start(
                        out=w_o, in_=d_wo[:].rearrange("(t p) o -> p t o", p=128)
                    )
                    for tt in range(TT):
                        l1pre = ln1p.tile([128, DIM], f32, tag="l1pre")
                        for nh in range(2):
                            ps = ps_o.tile([128, 384], f32, tag="op")
                            for kt in range(FT):
                                nc.tensor.matmul(
                                    ps,
                                    sb_zT[:, kt, tt * 128 : (tt + 1) * 128],
                                    w_o[:, kt, nh * 384 : (nh + 1) * 384],
                                    start=(kt == 0),
                                    stop=(kt == FT - 1),
                                )
                            nc.vector.scalar_tensor_tensor(
                                out=l1pre[:, nh * 384 : (nh + 1) * 384],
                                in0=ps,
                                scalar=1.0,
                                in1=bo_bc[:, nh * 384 : (nh + 1) * 384],
                                op0=ALU.mult,
                                op1=ALU.add,
                            )
                        xb1 = ln1p.tile([128, DIM], f32, tag="xb1")
                        nc.vector.tensor_add(xb1, sb_xblk[:, tt, :], bb1_bc)
                        layer_norm_to(sb_l1[:, tt, :], l1pre[:], g1_bc, xb1, ln1p)

            attn_res_cm.__exit__(None, None, None)
            sb_hT = big.tile([128, HT, BLK], bf16)  # relu(ffn1)^T, hid-major

            if MAX_PHASE >= 4:
                # ============ Phase 4: transpose l1, FFN1 ============
                with (
                    tc.tile_pool(name="w1_p", bufs=1) as w1_p,
                    tc.tile_pool(name="l1t_p", bufs=1) as l1t_p,
                    tc.tile_pool(name="ps_t", bufs=2, space="PSUM") as ps_t,
                    tc.tile_pool(name="ps_f1", bufs=4, space="PSUM") as ps_f1,
                ):
                    w1_t = []
                    for kt in range(FT):
                        wt = w1_p.tile([128, HID], bf16, tag=f"w1_{kt}")
                        nc.sync.dma_start(
                            out=wt, in_=d_w1[kt * 128 : (kt + 1) * 128, :]
                        )
                        w1_t.append(wt)
                    sb_l1T = l1t_p.tile([128, FT, BLK], bf16)
                    for ft in range(FT):
                        for tt in range(TT):
                            pst = ps_t.tile([128, 128], f32, tag="tp")
                            nc.tensor.transpose(
                                pst, sb_l1[:, tt, ft * 128 : (ft + 1) * 128], ident[:]
                            )
                            nc.scalar.copy(
                                sb_l1T[:, ft, tt * 128 : (tt + 1) * 128], pst
                            )
                    for ht2 in range(HT):
                        ps = ps_f1.tile([128, BLK], f32, tag="f1")
                        for kt in range(FT):
                            nc.tensor.matmul(
                                ps,
                                w1_t[kt][:, ht2 * 128 : (ht2 + 1) * 128],
                                sb_l1T[:, kt, :],
                                start=(kt == 0),
                                stop=(kt == FT - 1),
                            )
                        # relu(x + b1) on DVE: (x add b1) max 0
                        nc.vector.tensor_scalar(
                            sb_hT[:, ht2, :], ps, sb_b1[:, ht2 : ht2 + 1], 0.0,
                            op0=ALU.add, op1=ALU.max,
                        )

            if MAX_PHASE >= 5:
                # ============ Phase 5: FFN2 + LN2 + out ============
                with (
                    tc.tile_pool(name="w2_p", bufs=1) as w2_p,
                    tc.tile_pool(name="ln2p", bufs=2) as ln2p,
                    tc.tile_pool(name="outp", bufs=3) as outp,
                    tc.tile_pool(name="ps_f2", bufs=4, space="PSUM") as ps_f2,
                ):
                    w2_t = []
                    for kt in range(HT):
                        wt = w2_p.tile([128, DIM], bf16, tag=f"w2_{kt}")
                        nc.sync.dma_start(
                            out=wt, in_=d_w2[kt * 128 : (kt + 1) * 128, :]
                        )
                        w2_t.append(wt)
                    out_r = d_out[:].rearrange("(t p) d -> p t d", p=128)
                    for tt in range(TT):
                        f2pre = ln2p.tile([128, DIM], f32, tag="f2pre")
                        for nh in range(2):
                            ps = ps_f2.tile([128, 384], f32, tag="f2")
                            for kt in range(HT):
                                nc.tensor.matmul(
                                    ps,
                                    sb_hT[:, kt, tt * 128 : (tt + 1) * 128],
                                    w2_t[kt][:, nh * 384 : (nh + 1) * 384],
                                    start=(kt == 0),
                                    stop=(kt == HT - 1),
                                )
                            nc.vector.scalar_tensor_tensor(
                                out=f2pre[:, nh * 384 : (nh + 1) * 384],
                                in0=ps,
                                scalar=1.0,
                                in1=b2_bc[:, nh * 384 : (nh + 1) * 384],
                                op0=ALU.mult,
                                op1=ALU.add,
                            )
                        l1b = ln2p.tile([128, DIM], f32, tag="l1b")
                        nc.vector.tensor_add(l1b, sb_l1[:, tt, :], bb2_bc)
                        o_sb = outp.tile([128, DIM], f32, tag="osb")
                        layer_norm_to(o_sb[:], f2pre[:], g2_bc, l1b, ln2p)
                        nc.sync.dma_start(out=out_r[:, tt, :], in_=o_sb)

    return nc


def _get_nc(finalized=True):
    if "nc" not in _CACHE:
        _CACHE["nc"] = _build_program()
    nc = _CACHE["nc"]
    if finalized and not nc.is_finalized():
        nc.finalize()
    return nc


def make_in_maps(inputs: dict) -> list:
    x = np.asarray(inputs["x_n"], np.float32).reshape(B, S, DIM)
    mask = np.asarray(inputs["mask"]).reshape(B, S)
    w = {
        k: np.ascontiguousarray(np.asarray(inputs[k], np.float32).astype(BF16))
        for k in ("wq", "wk", "wv", "wo", "w1", "w2")
    }
    vecs = {
        "bq": inputs["bq"], "bk": inputs["bk"], "bv": inputs["bv"],
        "bo": inputs["bo"], "b1": inputs["b1"], "b2": inputs["b2"],
        "g1": inputs["ln1_g"], "bb1": inputs["ln1_b"],
        "g2": inputs["ln2_g"], "bb2": inputs["ln2_b"],
    }
    vecs = {k: np.ascontiguousarray(np.asarray(v, np.float32)) for k, v in vecs.items()}
    in_maps = []
    for c in range(N_CORES):
        b, blk = c // NBLK, c % NBLK
        xb = x[b]
        xT = None if USE_AG else np.ascontiguousarray(xb.T.astype(BF16))
        xblk = np.ascontiguousarray(xb[blk * BLK : (blk + 1) * BLK])
        xTb = np.ascontiguousarray(xblk.T.astype(BF16))
        msc = (mask[b].astype(np.float32) != 0).astype(np.float32) * ISCALE
        m = {"xTb": xTb, "xb": xblk, "msc": msc}
        if not USE_AG:
            m["xT"] = xT
        m.update(w)
        m.update(vecs)
        in_maps.append(m)
    return in_maps


def assemble(per_core_out: list) -> np.ndarray:
    blocks = [np.asarray(o, np.float32) for o in per_core_out]
    full = np.concatenate(blocks, axis=0).reshape(B, S, DIM)
    return full


def kernel(**inputs) -> np.ndarray:
    from concourse.bass_utils import run_bass_kernel_spmd

    nc = _get_nc()
    in_maps = make_in_maps(inputs)
    res = run_bass_kernel_spmd(nc, in_maps, list(range(N_CORES)))
    return assemble([r["out"] for r in res.results])



# revision 21
# speedup vs baseline: 1.1167x; 1.1167x over previous
"""Trainium2 Bass kernel for a dense transformer encoder layer.

Model (faithful to the oracle):
  q,k,v = x@wq+bq, x@wk+bk, x@wv+bv          (12 heads, dk=64, DIM=768)
  scores = q@k^T / sqrt(768)  (note: sqrt(dim_model), not sqrt(dk))
  scores[mask==0] = 1e-11  (NOT -inf; masked keys still contribute ~1/Z)
  attn = softmax(scores); z = attn@v; o = z@wo+bo
  l1 = x + LN(o);  ffn = relu(l1@w1+b1)@w2+b2;  out = l1 + LN(ffn)

Sharding: 4096 tokens (B=2,S=2048) split 8 ways -> 512 query tokens per
core. Cores 0-3 own batch 0, cores 4-7 batch 1. K/V are computed for
the core's whole batch (redundantly within each 4-core group) so there
are NO collectives: cores run fully independently, immune to cross-core
dispatch skew.

Mask trick: the key mask is folded into K at projection time:
K_masked[:,kpos] = (K[:,kpos]+bk) * m[kpos], m in {0,1}. Masked keys
produce scores == 0 exactly and exp(0) = 1.0 == fp32(exp(1e-11)),
matching the oracle bit-for-bit in fp32. The exp scale is then a
compile-time constant, so score tiles are exp'ed two PSUM banks
(1024 wide) per scalar-engine ACTIVATE.

Softmax denominator comes from a ones column appended to V (attn@v
with M=65); the reciprocal row is broadcast to 64 partitions with an
SBUF->SBUF stride-0 DMA and applied on the vector engine.

QKV projection and attention are streamed per head pair (= feature
tile of K^T/Q^T), so scalar-engine exps overlap tensor-engine work of
later pairs. FFN2 streams w2 in [128,384] slices with kt-outer chains
so it pipelines directly behind FFN1 tile production.
"""

import math
import os
import sys

import numpy as np

for _p in ("/opt/trn_rl_repo", os.path.expanduser("~/.axon_site/_ro/trn_rl_repo")):
    if os.path.isdir(_p) and _p not in sys.path:
        sys.path.insert(0, _p)

import ml_dtypes  # noqa: E402

BF16 = ml_dtypes.bfloat16

DIM = 768
HEADS = 12
DK = 64
HID = 4 * DIM  # 3072
B, S = 2, 2048
N_CORES = 8
BLK = 512            # query tokens per core
NBLK = S // BLK      # 4 blocks per batch
EPS = 1e-5
ISCALE = 1.0 / math.sqrt(DIM)

FT = DIM // 128   # 6 feature tiles (== head pairs)
TT = BLK // 128   # 4 token tiles per core block
ST = S // 128     # 16 key token tiles per batch
HT = HID // 128   # 24 hidden tiles

_CACHE: dict = {}
TAPS = os.environ.get("KERNEL_TAPS", "0") == "1"


def _build_program():
    import concourse.bass as bass
    import concourse.mybir as mybir
    import concourse.tile as tile
    from concourse import bacc
    from concourse.masks import make_identity

    f32 = mybir.dt.float32
    bf16 = mybir.dt.bfloat16
    AF = mybir.ActivationFunctionType
    ALU = mybir.AluOpType
    AX = mybir.AxisListType

    nc = bacc.Bacc()

    # ---- per-core DRAM I/O ----
    d_xT = nc.dram_tensor("xT", [DIM, S], bf16, kind="ExternalInput")
    d_xTb = nc.dram_tensor("xTb", [DIM, BLK], bf16, kind="ExternalInput")
    d_xb = nc.dram_tensor("xb", [BLK, DIM], f32, kind="ExternalInput")
    d_msk = nc.dram_tensor("msk", [S], f32, kind="ExternalInput")
    d_wq = nc.dram_tensor("wq", [DIM, DIM], bf16, kind="ExternalInput")
    d_wk = nc.dram_tensor("wk", [DIM, DIM], bf16, kind="ExternalInput")
    d_wv = nc.dram_tensor("wv", [DIM, DIM], bf16, kind="ExternalInput")
    d_wo = nc.dram_tensor("wo", [DIM, DIM], bf16, kind="ExternalInput")
    d_w1 = nc.dram_tensor("w1", [DIM, HID], bf16, kind="ExternalInput")
    d_w2 = nc.dram_tensor("w2", [HID, DIM], bf16, kind="ExternalInput")
    d_bq = nc.dram_tensor("bq", [DIM], f32, kind="ExternalInput")
    d_bk = nc.dram_tensor("bk", [DIM], f32, kind="ExternalInput")
    d_bv = nc.dram_tensor("bv", [DIM], f32, kind="ExternalInput")
    d_bo = nc.dram_tensor("bo", [DIM], f32, kind="ExternalInput")
    d_b1 = nc.dram_tensor("b1", [HID], f32, kind="ExternalInput")
    d_b2 = nc.dram_tensor("b2", [DIM], f32, kind="ExternalInput")
    d_g1 = nc.dram_tensor("g1", [DIM], f32, kind="ExternalInput")
    d_bb1 = nc.dram_tensor("bb1", [DIM], f32, kind="ExternalInput")
    d_g2 = nc.dram_tensor("g2", [DIM], f32, kind="ExternalInput")
    d_bb2 = nc.dram_tensor("bb2", [DIM], f32, kind="ExternalInput")
    d_out = nc.dram_tensor("out", [BLK, DIM], f32, kind="ExternalOutput")
    if TAPS:
        bf16_ = __import__("concourse.mybir", fromlist=["dt"]).dt.bfloat16
        d_tap_K = nc.dram_tensor("tap_K", [128, FT, S], bf16_, kind="ExternalOutput")
        d_tap_Q = nc.dram_tensor("tap_Q", [128, FT, BLK], bf16_, kind="ExternalOutput")
        d_tap_V = nc.dram_tensor(
            "tap_V", [128, ST, HEADS, DK + 1], bf16_, kind="ExternalOutput"
        )
        d_tap_zT = nc.dram_tensor("tap_zT", [128, FT, BLK], bf16_, kind="ExternalOutput")
        d_tap_l1 = nc.dram_tensor("tap_l1", [128, TT, DIM], f32, kind="ExternalOutput")
        d_tap_et = nc.dram_tensor("tap_et", [128, 2, BLK], bf16_, kind="ExternalOutput")
        d_tap_rs = nc.dram_tensor("tap_rs", [1, BLK], f32, kind="ExternalOutput")

    def bcast_ap(handle, n=128):
        ap = handle[:]
        return bass.AP(tensor=ap.tensor, offset=ap.offset, ap=[[0, n]] + list(ap.ap))

    def bcast_sb(ap, n):
        # partition-stride-0 view of a [1, N] SBUF AP, for DMA broadcast
        return bass.AP(tensor=ap.tensor, offset=ap.offset, ap=[[0, n]] + list(ap.ap)[1:])

    with tile.TileContext(nc) as tc:
        with (
            tc.tile_pool(name="const", bufs=1) as const,
            tc.tile_pool(name="bigres", bufs=1) as big,
        ):
            # ---------- constants ----------
            sb_bq = const.tile([128, FT], f32)
            nc.sync.dma_start(out=sb_bq, in_=d_bq[:].rearrange("(t p) -> p t", p=128))
            sb_bk = const.tile([128, FT], f32)
            nc.sync.dma_start(out=sb_bk, in_=d_bk[:].rearrange("(t p) -> p t", p=128))
            sb_b1 = const.tile([128, HT], f32)
            nc.sync.dma_start(out=sb_b1, in_=d_b1[:].rearrange("(t p) -> p t", p=128))
            mask_bc = const.tile([128, S], f32)
            nc.gpsimd.dma_start(out=mask_bc, in_=bcast_ap(d_msk))
            bv_bc = const.tile([128, DIM], f32)
            nc.gpsimd.dma_start(out=bv_bc, in_=bcast_ap(d_bv))
            bo_bc = const.tile([128, DIM], f32)
            nc.gpsimd.dma_start(out=bo_bc, in_=bcast_ap(d_bo))
            b2_bc = const.tile([128, DIM], f32)
            nc.gpsimd.dma_start(out=b2_bc, in_=bcast_ap(d_b2))
            g1_bc = const.tile([128, DIM], f32)
            nc.gpsimd.dma_start(out=g1_bc, in_=bcast_ap(d_g1))
            bb1_bc = const.tile([128, DIM], f32)
            nc.gpsimd.dma_start(out=bb1_bc, in_=bcast_ap(d_bb1))
            g2_bc = const.tile([128, DIM], f32)
            nc.gpsimd.dma_start(out=g2_bc, in_=bcast_ap(d_g2))
            bb2_bc = const.tile([128, DIM], f32)
            nc.gpsimd.dma_start(out=bb2_bc, in_=bcast_ap(d_bb2))
            ident = const.tile([128, 128], f32)
            make_identity(nc, ident[:])
            ones64 = const.tile([1, 64], f32)
            nc.vector.memset(ones64, 1.0)
            eps_t = const.tile([128, 1], f32)
            nc.vector.memset(eps_t, EPS)

            # persistent across attention->FFN boundary
            sb_l1 = big.tile([128, TT, DIM], f32)
            sb_l1T = big.tile([128, FT, BLK], bf16)

            # ---- attention residents (die after O-proj/LN1) ----
            attn_cm = tc.tile_pool(name="attn_res", bufs=1)
            attn_res = attn_cm.__enter__()
            sb_K = attn_res.tile([128, FT, S], bf16)    # K^T feat-major, masked
            sb_Q = attn_res.tile([128, FT, BLK], bf16)  # Q^T feat-major
            sb_V = attn_res.tile([128, ST, HEADS, DK + 1], bf16)  # V + ones col
            sb_zT = attn_res.tile([128, FT, BLK], bf16)  # z^T normalized

            wo_cm = tc.tile_pool(name="wo_p", bufs=1)
            wo_p = wo_cm.__enter__()
            w_o = wo_p.tile([128, FT, DIM], bf16)
            nc.sync.dma_start(out=w_o, in_=d_wo[:].rearrange("(t p) o -> p t o", p=128))

            # ---- QKV-phase residents (die after last projection) ----
            xt_cm = tc.tile_pool(name="xt_p", bufs=1)
            xt_p = xt_cm.__enter__()
            sb_xT = xt_p.tile([128, FT, S], bf16)
            nc.sync.dma_start(
                out=sb_xT, in_=d_xT[:].rearrange("(t p) n -> p t n", p=128)
            )
            sb_xTb = xt_p.tile([128, FT, BLK], bf16)
            nc.sync.dma_start(
                out=sb_xTb, in_=d_xTb[:].rearrange("(t p) n -> p t n", p=128)
            )
            wqkv_cm = tc.tile_pool(name="wqkv_p", bufs=1)
            wqkv_p = wqkv_cm.__enter__()
            w_k = wqkv_p.tile([128, FT, DIM], bf16)
            nc.sync.dma_start(out=w_k, in_=d_wk[:].rearrange("(t p) o -> p t o", p=128))
            w_q = wqkv_p.tile([128, FT, DIM], bf16)
            nc.sync.dma_start(out=w_q, in_=d_wq[:].rearrange("(t p) o -> p t o", p=128))
            w_v = wqkv_p.tile([128, FT, DIM], bf16)
            nc.sync.dma_start(out=w_v, in_=d_wv[:].rearrange("(t p) o -> p t o", p=128))

            # ============ QKV + attention, streamed per head pair ============
            ets_cm = tc.tile_pool(name="ets", bufs=3)
            ets_p = ets_cm.__enter__()
            attsm_cm = tc.tile_pool(name="attsm", bufs=2)
            attsm = attsm_cm.__enter__()
            ps_qkv_cm = tc.tile_pool(name="ps_qkv", bufs=2, space="PSUM")
            ps_qkv = ps_qkv_cm.__enter__()
            ps_sc_cm = tc.tile_pool(name="ps_sc", bufs=2, space="PSUM")
            ps_sc = ps_sc_cm.__enter__()
            ps_z_cm = tc.tile_pool(name="ps_z", bufs=1, space="PSUM")
            ps_z = ps_z_cm.__enter__()
            ps_rb_cm = tc.tile_pool(name="ps_rb", bufs=1, space="PSUM")
            ps_rb = ps_rb_cm.__enter__()

            nc.vector.memset(sb_V[:, :, :, DK : DK + 1], 1.0)

            for ft in range(FT):
                # K^T[ft] over the whole batch, bias + mask folded in
                for nt in range(S // 512):
                    ps = ps_qkv.tile([128, 512], f32, tag="p")
                    for kt in range(FT):
                        nc.tensor.matmul(
                            ps,
                            w_k[:, kt, ft * 128 : (ft + 1) * 128],
                            sb_xT[:, kt, nt * 512 : (nt + 1) * 512],
                            start=(kt == 0),
                            stop=(kt == FT - 1),
                        )
                    nc.vector.scalar_tensor_tensor(
                        out=sb_K[:, ft, nt * 512 : (nt + 1) * 512],
                        in0=ps,
                        scalar=sb_bk[:, ft : ft + 1],
                        in1=mask_bc[:, nt * 512 : (nt + 1) * 512],
                        op0=ALU.add,
                        op1=ALU.mult,
                    )
                # Q^T[ft] for the core's own block
                ps = ps_qkv.tile([128, 512], f32, tag="p")
                for kt in range(FT):
                    nc.tensor.matmul(
                        ps,
                        w_q[:, kt, ft * 128 : (ft + 1) * 128],
                        sb_xTb[:, kt, :],
                        start=(kt == 0),
                        stop=(kt == FT - 1),
                    )
                nc.vector.tensor_scalar_add(sb_Q[:, ft, :], ps, sb_bq[:, ft : ft + 1])

                if ft == 0:
                    # V tok-major over the whole batch, laid out [tok, head, dk+1].
                    # Must be complete before the first z matmul below.
                    for nh in range(2):
                        for tt2 in range(ST):
                            psv = ps_qkv.tile([128, 512], f32, tag="p")
                            for kt in range(FT):
                                nc.tensor.matmul(
                                    psv[:, 0:384],
                                    sb_xT[:, kt, tt2 * 128 : (tt2 + 1) * 128],
                                    w_v[:, kt, nh * 384 : (nh + 1) * 384],
                                    start=(kt == 0),
                                    stop=(kt == FT - 1),
                                )
                            nc.vector.scalar_tensor_tensor(
                                out=sb_V[:, tt2, nh * 6 : (nh + 1) * 6, 0:DK],
                                in0=psv[:, 0:384].rearrange("p (h d) -> p h d", d=DK),
                                scalar=1.0,
                                in1=bv_bc[:, nh * 384 : (nh + 1) * 384].rearrange(
                                    "p (h d) -> p h d", d=DK
                                ),
                                op0=ALU.mult,
                                op1=ALU.add,
                            )

                # scores + exp + z for the two heads of this feature tile
                for half in (0, 1):
                    h = 2 * ft + half
                    ho = half * 64
                    zp = ps_z.tile([DK + 1, BLK], f32, tag="z")
                    for b8 in range(ST // 2):
                        pssc = ps_sc.tile([128, 2, 512], f32, tag="sc")
                        for j in (0, 1):
                            kt2 = b8 * 2 + j
                            nc.tensor.matmul(
                                pssc[:, j, :],
                                sb_K[ho : ho + 64, ft, kt2 * 128 : (kt2 + 1) * 128],
                                sb_Q[ho : ho + 64, ft, :],
                                start=True,
                                stop=True,
                            )
                        et = ets_p.tile([128, 2, BLK], bf16, tag="exp")
                        nc.scalar.activation(
                            et[:].rearrange("p a b -> p (a b)"),
                            pssc[:].rearrange("p a b -> p (a b)"),
                            AF.Exp,
                            scale=ISCALE,
                        )
                        if TAPS and h == 0 and b8 == 0:
                            nc.sync.dma_start(out=d_tap_et[:], in_=et)
                        for j in (0, 1):
                            kt2 = b8 * 2 + j
                            nc.tensor.matmul(
                                zp,
                                sb_V[:, kt2, h, :],
                                et[:, j, :],
                                start=(kt2 == 0),
                                stop=(kt2 == ST - 1),
                            )
                    # normalize: zT = z[:64] * (1/sum), sum in row 64
                    rsum = attsm.tile([1, BLK], f32, tag="rsum")
                    nc.vector.reciprocal(rsum, zp[DK : DK + 1, :])
                    if TAPS and h == 0:
                        nc.sync.dma_start(out=d_tap_rs[:], in_=rsum)
                    rbp = ps_rb.tile([64, BLK], f32, tag="rb")
                    nc.tensor.matmul(rbp, ones64[:], rsum, start=True, stop=True)
                    rb = attsm.tile([64, BLK], f32, tag="rbs")
                    nc.vector.tensor_copy(rb, rbp)
                    nc.vector.tensor_mul(
                        sb_zT[ho : ho + 64, ft, :], zp[0:DK, :], rb
                    )

            if TAPS:
                nc.sync.dma_start(out=d_tap_K[:], in_=sb_K)
                nc.sync.dma_start(out=d_tap_Q[:], in_=sb_Q)
                nc.sync.dma_start(out=d_tap_V[:], in_=sb_V)
                nc.sync.dma_start(out=d_tap_zT[:], in_=sb_zT)

            ps_rb_cm.__exit__(None, None, None)
            ps_z_cm.__exit__(None, None, None)
            ps_sc_cm.__exit__(None, None, None)
            ps_qkv_cm.__exit__(None, None, None)
            attsm_cm.__exit__(None, None, None)
            ets_cm.__exit__(None, None, None)
            wqkv_cm.__exit__(None, None, None)
            xt_cm.__exit__(None, None, None)

            # ============ O proj + LN1 (+residual) + l1 transpose ============
            def layer_norm_to(out_ap, x_ap, g_bc_t, resid_ap, pool):
                s = pool.tile([128, 1], f32, tag="ln_s")
                nc.vector.tensor_reduce(s, x_ap, axis=AX.X, op=ALU.add)
                mean = pool.tile([128, 1], f32, tag="ln_m")
                nc.vector.tensor_scalar_mul(mean, s, 1.0 / DIM)
                xc = pool.tile([128, DIM], f32, tag="ln_xc")
                nc.vector.tensor_scalar(xc, x_ap, mean, None, op0=ALU.subtract)
                junk = pool.tile([128, DIM], bf16, tag="ln_j")
                var = pool.tile([128, 1], f32, tag="ln_v")
                nc.scalar.activation(junk, xc, AF.Square, accum_out=var)
                sd = pool.tile([128, 1], f32, tag="ln_sd")
                nc.scalar.activation(sd, var, AF.Sqrt, bias=eps_t[:], scale=1.0 / DIM)
                rstd = pool.tile([128, 1], f32, tag="ln_r")
                nc.vector.reciprocal(rstd, sd)
                tg = pool.tile([128, DIM], f32, tag="ln_tg")
                nc.vector.scalar_tensor_tensor(
                    out=tg, in0=xc, scalar=rstd, in1=g_bc_t,
                    op0=ALU.mult, op1=ALU.mult,
                )
                nc.vector.tensor_add(out_ap, tg, resid_ap)

            xb_r = d_xb[:].rearrange("(t p) d -> p t d", p=128)
            with (
                tc.tile_pool(name="ln1p", bufs=2) as ln1p,
                tc.tile_pool(name="ps_o", bufs=2, space="PSUM") as ps_o,
                tc.tile_pool(name="ps_t", bufs=2, space="PSUM") as ps_t,
            ):
                for tt in range(TT):
                    l1pre = ln1p.tile([128, DIM], f32, tag="l1pre")
                    for nh in range(2):
                        ps = ps_o.tile([128, 384], f32, tag="op")
                        for kt in range(FT):
                            nc.tensor.matmul(
                                ps,
                                sb_zT[:, kt, tt * 128 : (tt + 1) * 128],
                                w_o[:, kt, nh * 384 : (nh + 1) * 384],
                                start=(kt == 0),
                                stop=(kt == FT - 1),
                            )
                        nc.vector.scalar_tensor_tensor(
                            out=l1pre[:, nh * 384 : (nh + 1) * 384],
                            in0=ps,
                            scalar=1.0,
                            in1=bo_bc[:, nh * 384 : (nh + 1) * 384],
                            op0=ALU.mult,
                            op1=ALU.add,
                        )
                    xbt = ln1p.tile([128, DIM], f32, tag="xbt")
                    nc.sync.dma_start(out=xbt, in_=xb_r[:, tt, :])
                    xb1 = ln1p.tile([128, DIM], f32, tag="xb1")
                    nc.vector.tensor_add(xb1, xbt, bb1_bc)
                    layer_norm_to(sb_l1[:, tt, :], l1pre[:], g1_bc, xb1, ln1p)
                    # transpose l1[tt] right away so FFN1 can start early
                    for ft in range(FT):
                        pst = ps_t.tile([128, 128], f32, tag="tp")
                        nc.tensor.transpose(
                            pst, sb_l1[:, tt, ft * 128 : (ft + 1) * 128], ident[:]
                        )
                        nc.vector.tensor_copy(
                            sb_l1T[:, ft, tt * 128 : (tt + 1) * 128], pst
                        )

            if TAPS:
                nc.sync.dma_start(out=d_tap_l1[:], in_=sb_l1)

            wo_cm.__exit__(None, None, None)
            attn_cm.__exit__(None, None, None)

            # ============ FFN1 -> hT, FFN2 streamed behind it ============
            w1_cm = tc.tile_pool(name="w1_p", bufs=1)
            w1_p = w1_cm.__enter__()
            w_1 = w1_p.tile([128, FT, HID], bf16)
            nc.sync.dma_start(out=w_1, in_=d_w1[:].rearrange("(t p) o -> p t o", p=128))
            hT_cm = tc.tile_pool(name="hT_p", bufs=1)
            hT_p = hT_cm.__enter__()
            sb_hT = hT_p.tile([128, HT, BLK], bf16)  # relu(ffn1)^T, hid-major

            with (
                tc.tile_pool(name="w2s", bufs=6) as w2s_p,
                tc.tile_pool(name="ln2p", bufs=2) as ln2p,
                tc.tile_pool(name="f2pre_p", bufs=4) as f2pre_p,
                tc.tile_pool(name="outp", bufs=3) as outp,
                tc.tile_pool(name="ps_f1", bufs=2, space="PSUM") as ps_f1,
                tc.tile_pool(name="ps_f2", bufs=4, space="PSUM") as ps_f2,
            ):
                for ht2 in range(HT):
                    ps = ps_f1.tile([128, BLK], f32, tag="f1")
                    for kt in range(FT):
                        nc.tensor.matmul(
                            ps,
                            w_1[:, kt, ht2 * 128 : (ht2 + 1) * 128],
                            sb_l1T[:, kt, :],
                            start=(kt == 0),
                            stop=(kt == FT - 1),
                        )
                    # relu(x + b1) on DVE: (x add b1) max 0
                    nc.vector.tensor_scalar(
                        sb_hT[:, ht2, :], ps, sb_b1[:, ht2 : ht2 + 1], 0.0,
                        op0=ALU.add, op1=ALU.max,
                    )

                f2pre = []
                for _tt in range(TT):
                    f2pre_t = f2pre_p.tile([128, DIM], f32, tag="f2pre")
                    f2pre.append(f2pre_t)
                for nh in range(2):
                    chains = []
                    for _tt in range(TT):
                        chain_t = ps_f2.tile([128, 384], f32, tag="f2")
                        chains.append(chain_t)
                    for kt in range(HT):
                        w2s = w2s_p.tile([128, 384], bf16, tag="w2s")
                        nc.sync.dma_start(
                            out=w2s,
                            in_=d_w2[kt * 128 : (kt + 1) * 128,
                                     nh * 384 : (nh + 1) * 384],
                        )
                        for tt in range(TT):
                            nc.tensor.matmul(
                                chains[tt],
                                sb_hT[:, kt, tt * 128 : (tt + 1) * 128],
                                w2s,
                                start=(kt == 0),
                                stop=(kt == HT - 1),
                            )
                    for tt in range(TT):
                        nc.vector.scalar_tensor_tensor(
                            out=f2pre[tt][:, nh * 384 : (nh + 1) * 384],
                            in0=chains[tt],
                            scalar=1.0,
                            in1=b2_bc[:, nh * 384 : (nh + 1) * 384],
                            op0=ALU.mult,
                            op1=ALU.add,
                        )

                out_r = d_out[:].rearrange("(t p) d -> p t d", p=128)
                for tt in range(TT):
                    l1b = ln2p.tile([128, DIM], f32, tag="l1b")
                    nc.vector.tensor_add(l1b, sb_l1[:, tt, :], bb2_bc)
                    o_sb = outp.tile([128, DIM], f32, tag="osb")
                    layer_norm_to(o_sb[:], f2pre[tt][:], g2_bc, l1b, ln2p)
                    nc.sync.dma_start(out=out_r[:, tt, :], in_=o_sb)

            hT_cm.__exit__(None, None, None)
            w1_cm.__exit__(None, None, None)

    return nc


def _get_nc(finalized=True):
    if "nc" not in _CACHE:
        _CACHE["nc"] = _build_program()
    nc = _CACHE["nc"]
    if finalized and not nc.is_finalized():
        nc.finalize()
    return nc


def make_in_maps(inputs: dict) -> list:
    x = np.asarray(inputs["x_n"], np.float32).reshape(B, S, DIM)
    mask = np.asarray(inputs["mask"]).reshape(B, S)
    w = {
        k: np.ascontiguousarray(np.asarray(inputs[k], np.float32).astype(BF16))
        for k in ("wq", "wk", "wv", "wo", "w1", "w2")
    }
    vecs = {
        "bq": inputs["bq"], "bk": inputs["bk"], "bv": inputs["bv"],
        "bo": inputs["bo"], "b1": inputs["b1"], "b2": inputs["b2"],
        "g1": inputs["ln1_g"], "bb1": inputs["ln1_b"],
        "g2": inputs["ln2_g"], "bb2": inputs["ln2_b"],
    }
    vecs = {k: np.ascontiguousarray(np.asarray(v, np.float32)) for k, v in vecs.items()}
    in_maps = []
    for c in range(N_CORES):
        b, blk = c // NBLK, c % NBLK
        xb_full = x[b]
        xT = np.ascontiguousarray(xb_full.T.astype(BF16))
        xblk = np.ascontiguousarray(xb_full[blk * BLK : (blk + 1) * BLK])
        xTb = np.ascontiguousarray(xblk.T.astype(BF16))
        msk = (mask[b] != 0).astype(np.float32)
        m = {"xT": xT, "xTb": xTb, "xb": xblk, "msk": msk}
        m.update(w)
        m.update(vecs)
        in_maps.append(m)
    return in_maps


def assemble(per_core_out: list) -> np.ndarray:
    blocks = [np.asarray(o, np.float32) for o in per_core_out]
    full = np.concatenate(blocks, axis=0).reshape(B, S, DIM)
    return full


def kernel(**inputs) -> np.ndarray:
    from concourse.bass_utils import run_bass_kernel_spmd

    nc = _get_nc()
    in_maps = make_in_maps(inputs)
    res = run_bass_kernel_spmd(nc, in_maps, list(range(N_CORES)))
    return assemble([r["out"] for r in res.results])


# revision 24
# speedup vs baseline: 1.1535x; 1.0329x over previous
"""Trainium2 Bass kernel for a dense transformer encoder layer.

Model (faithful to the oracle):
  q,k,v = x@wq+bq, x@wk+bk, x@wv+bv          (12 heads, dk=64, DIM=768)
  scores = q@k^T / sqrt(768)  (note: sqrt(dim_model), not sqrt(dk))
  scores[mask==0] = 1e-11  (NOT -inf; masked keys still contribute ~1/Z)
  attn = softmax(scores); z = attn@v; o = z@wo+bo
  l1 = x + LN(o);  ffn = relu(l1@w1+b1)@w2+b2;  out = l1 + LN(ffn)

Sharding: 4096 tokens (B=2,S=2048) split 8 ways -> 512 query tokens per
core. Cores 0-3 own batch 0, cores 4-7 batch 1. K/V are computed for
the core's whole batch (redundantly within each 4-core group) so there
are NO collectives: cores run fully independently, immune to cross-core
dispatch skew.

Mask trick: the key mask is folded into K at projection time:
K_masked[:,kpos] = (K[:,kpos]+bk) * m[kpos], m in {0,1}. Masked keys
produce scores == 0 exactly and exp(0) = 1.0 == fp32(exp(1e-11)),
matching the oracle bit-for-bit in fp32. The exp scale is then a
compile-time constant, so score tiles are exp'ed two PSUM banks
(1024 wide) per scalar-engine ACTIVATE.

Softmax denominator comes from a ones column appended to V (attn@v
with M=65). The per-head normalization tail (reciprocal -> rank-1
broadcast matmul -> multiply) is deferred past the next head's score
matmuls so the in-order PE queue never stalls on the DVE reciprocal;
the broadcast lands in the unused partitions 64..127 of the same PSUM
bank as z.

All multi-MB DMAs are split into per-feature-tile chunks and issued
round-robin over the three DMA-capable queues (sync/scalar/gpsimd) —
a single dma_start rides one DMA engine at ~26 GB/s, so chunking is
what buys parallel HBM bandwidth.
"""

import math
import os
import sys

import numpy as np

for _p in ("/opt/trn_rl_repo", os.path.expanduser("~/.axon_site/_ro/trn_rl_repo")):
    if os.path.isdir(_p) and _p not in sys.path:
        sys.path.insert(0, _p)

import ml_dtypes  # noqa: E402

BF16 = ml_dtypes.bfloat16

DIM = 768
HEADS = 12
DK = 64
HID = 4 * DIM  # 3072
B, S = 2, 2048
N_CORES = 8
BLK = 512            # query tokens per core
NBLK = S // BLK      # 4 blocks per batch
EPS = 1e-5
ISCALE = 1.0 / math.sqrt(DIM)

FT = DIM // 128   # 6 feature tiles (== head pairs)
TT = BLK // 128   # 4 token tiles per core block
ST = S // 128     # 16 key token tiles per batch
HT = HID // 128   # 24 hidden tiles

_CACHE: dict = {}
TAPS = os.environ.get("KERNEL_TAPS", "0") == "1"


def _build_program():
    import concourse.bass as bass
    import concourse.mybir as mybir
    import concourse.tile as tile
    from concourse import bacc
    from concourse.masks import make_identity

    f32 = mybir.dt.float32
    bf16 = mybir.dt.bfloat16
    AF = mybir.ActivationFunctionType
    ALU = mybir.AluOpType
    AX = mybir.AxisListType

    nc = bacc.Bacc()

    # ---- per-core DRAM I/O ----
    d_xT = nc.dram_tensor("xT", [DIM, S], bf16, kind="ExternalInput")
    d_xTb = nc.dram_tensor("xTb", [DIM, BLK], bf16, kind="ExternalInput")
    d_xb = nc.dram_tensor("xb", [BLK, DIM], f32, kind="ExternalInput")
    d_msk = nc.dram_tensor("msk", [S], f32, kind="ExternalInput")
    d_wq = nc.dram_tensor("wq", [DIM, DIM], bf16, kind="ExternalInput")
    d_wk = nc.dram_tensor("wk", [DIM, DIM], bf16, kind="ExternalInput")
    d_wv = nc.dram_tensor("wv", [DIM, DIM], bf16, kind="ExternalInput")
    d_wo = nc.dram_tensor("wo", [DIM, DIM], bf16, kind="ExternalInput")
    d_w1 = nc.dram_tensor("w1", [DIM, HID], bf16, kind="ExternalInput")
    d_w2 = nc.dram_tensor("w2", [HID, DIM], bf16, kind="ExternalInput")
    d_bq = nc.dram_tensor("bq", [DIM], f32, kind="ExternalInput")
    d_bk = nc.dram_tensor("bk", [DIM], f32, kind="ExternalInput")
    d_bv = nc.dram_tensor("bv", [DIM], f32, kind="ExternalInput")
    d_bo = nc.dram_tensor("bo", [DIM], f32, kind="ExternalInput")
    d_b1 = nc.dram_tensor("b1", [HID], f32, kind="ExternalInput")
    d_b2 = nc.dram_tensor("b2", [DIM], f32, kind="ExternalInput")
    d_g1 = nc.dram_tensor("g1", [DIM], f32, kind="ExternalInput")
    d_bb1 = nc.dram_tensor("bb1", [DIM], f32, kind="ExternalInput")
    d_g2 = nc.dram_tensor("g2", [DIM], f32, kind="ExternalInput")
    d_bb2 = nc.dram_tensor("bb2", [DIM], f32, kind="ExternalInput")
    d_out = nc.dram_tensor("out", [BLK, DIM], f32, kind="ExternalOutput")
    if TAPS:
        d_tap_zT = nc.dram_tensor("tap_zT", [128, FT, BLK], bf16, kind="ExternalOutput")
        d_tap_l1 = nc.dram_tensor("tap_l1", [128, TT, DIM], f32, kind="ExternalOutput")

    def bcast_ap(handle, n=128):
        ap = handle[:]
        return bass.AP(tensor=ap.tensor, offset=ap.offset, ap=[[0, n]] + list(ap.ap))

    with tile.TileContext(nc) as tc:
        with (
            tc.tile_pool(name="const", bufs=1) as const,
            tc.tile_pool(name="bigres", bufs=1) as big,
        ):
            # issue-queue round-robin for chunked DMAs
            _eng = [nc.sync, nc.scalar, nc.gpsimd]

            # ---------- constants ----------
            # mask first on gpsimd: the first K-copy needs it
            mask_bc = const.tile([128, S], f32)
            nc.gpsimd.dma_start(out=mask_bc, in_=bcast_ap(d_msk))
            sb_bq = const.tile([128, FT], f32)
            nc.sync.dma_start(out=sb_bq, in_=d_bq[:].rearrange("(t p) -> p t", p=128))
            sb_bk = const.tile([128, FT], f32)
            nc.sync.dma_start(out=sb_bk, in_=d_bk[:].rearrange("(t p) -> p t", p=128))
            sb_b1 = const.tile([128, HT], f32)
            nc.sync.dma_start(out=sb_b1, in_=d_b1[:].rearrange("(t p) -> p t", p=128))
            ident = const.tile([128, 128], f32)
            make_identity(nc, ident[:])
            ones64 = const.tile([1, 64], f32)
            nc.vector.memset(ones64, 1.0)
            eps_t = const.tile([128, 1], f32)
            nc.vector.memset(eps_t, EPS)

            # persistent across attention->FFN boundary
            sb_l1 = big.tile([128, TT, DIM], f32)
            sb_l1T = big.tile([128, FT, BLK], bf16)

            # ---- attention residents (die after O-proj/LN1) ----
            attn_cm = tc.tile_pool(name="attn_res", bufs=1)
            attn_res = attn_cm.__enter__()
            sb_K = attn_res.tile([128, FT, S], bf16)    # K^T feat-major, masked
            sb_Q = attn_res.tile([128, FT, BLK], bf16)  # Q^T feat-major
            sb_V = attn_res.tile([128, ST, HEADS, DK + 1], bf16)  # V + ones col
            sb_zT = attn_res.tile([128, FT, BLK], bf16)  # z^T normalized

            wo_cm = tc.tile_pool(name="wo_p", bufs=1)
            wo_p = wo_cm.__enter__()

            # ---- QKV-phase residents, chunked per feature tile ----
            xt_cm = tc.tile_pool(name="xt_p", bufs=1)
            xt_p = xt_cm.__enter__()
            wqkv_cm = tc.tile_pool(name="wqkv_p", bufs=1)
            wqkv_p = wqkv_cm.__enter__()

            wk_t, xt_t = [], []
            for kt in range(FT):  # interleave: wk + xT are needed first
                wkt = wqkv_p.tile([128, DIM], bf16, tag=f"wk{kt}")
                _eng[kt % 3].dma_start(
                    out=wkt, in_=d_wk[kt * 128 : (kt + 1) * 128, :]
                )
                wk_t.append(wkt)
                xtt = xt_p.tile([128, S], bf16, tag=f"xt{kt}")
                _eng[(kt + 1) % 3].dma_start(
                    out=xtt, in_=d_xT[kt * 128 : (kt + 1) * 128, :]
                )
                xt_t.append(xtt)
            xtb_t, wq_t = [], []
            for kt in range(FT):
                xtbt = xt_p.tile([128, BLK], bf16, tag=f"xtb{kt}")
                _eng[kt % 3].dma_start(
                    out=xtbt, in_=d_xTb[kt * 128 : (kt + 1) * 128, :]
                )
                xtb_t.append(xtbt)
                wqt = wqkv_p.tile([128, DIM], bf16, tag=f"wq{kt}")
                _eng[(kt + 1) % 3].dma_start(
                    out=wqt, in_=d_wq[kt * 128 : (kt + 1) * 128, :]
                )
                wq_t.append(wqt)
            wv_t = []
            for kt in range(FT):
                wvt = wqkv_p.tile([128, DIM], bf16, tag=f"wv{kt}")
                _eng[kt % 3].dma_start(
                    out=wvt, in_=d_wv[kt * 128 : (kt + 1) * 128, :]
                )
                wv_t.append(wvt)
            # remaining bcast consts (gpsimd) after the critical-path loads
            bv_bc = const.tile([128, DIM], f32)
            nc.gpsimd.dma_start(out=bv_bc, in_=bcast_ap(d_bv))
            bo_bc = const.tile([128, DIM], f32)
            nc.gpsimd.dma_start(out=bo_bc, in_=bcast_ap(d_bo))
            b2_bc = const.tile([128, DIM], f32)
            nc.gpsimd.dma_start(out=b2_bc, in_=bcast_ap(d_b2))
            g1_bc = const.tile([128, DIM], f32)
            nc.gpsimd.dma_start(out=g1_bc, in_=bcast_ap(d_g1))
            bb1_bc = const.tile([128, DIM], f32)
            nc.gpsimd.dma_start(out=bb1_bc, in_=bcast_ap(d_bb1))
            g2_bc = const.tile([128, DIM], f32)
            nc.gpsimd.dma_start(out=g2_bc, in_=bcast_ap(d_g2))
            bb2_bc = const.tile([128, DIM], f32)
            nc.gpsimd.dma_start(out=bb2_bc, in_=bcast_ap(d_bb2))
            wo_t = []
            for kt in range(FT):
                wot = wo_p.tile([128, DIM], bf16, tag=f"wo{kt}")
                _eng[kt % 3].dma_start(
                    out=wot, in_=d_wo[kt * 128 : (kt + 1) * 128, :]
                )
                wo_t.append(wot)

            # ============ QKV + attention, streamed per head pair ============
            ets_cm = tc.tile_pool(name="ets", bufs=3)
            ets_p = ets_cm.__enter__()
            attsm_cm = tc.tile_pool(name="attsm", bufs=2)
            attsm = attsm_cm.__enter__()
            ps_qkv_cm = tc.tile_pool(name="ps_qkv", bufs=2, space="PSUM")
            ps_qkv = ps_qkv_cm.__enter__()
            ps_sc_cm = tc.tile_pool(name="ps_sc", bufs=2, space="PSUM")
            ps_sc = ps_sc_cm.__enter__()
            ps_z_cm = tc.tile_pool(name="ps_z", bufs=2, space="PSUM")
            ps_z = ps_z_cm.__enter__()

            nc.vector.memset(sb_V[:, :, :, DK : DK + 1], 1.0)

            # deferred normalization tail: (zp, rsum, ho, ft)
            pending: list = []

            def flush_tail():
                if not pending:
                    return
                zp, rsum, p_ho, p_ft = pending.pop()
                rbp = zp[DK : DK + DK, :]
                nc.tensor.matmul(rbp, ones64[:], rsum, start=True, stop=True)
                rb = attsm.tile([64, BLK], f32, tag="rbs")
                nc.vector.tensor_copy(rb, rbp)
                nc.vector.tensor_mul(
                    sb_zT[p_ho : p_ho + 64, p_ft, :], zp[0:DK, :], rb
                )

            for ft in range(FT):
                # K^T[ft] over the whole batch, bias + mask folded in
                for nt in range(S // 512):
                    ps = ps_qkv.tile([128, 512], f32, tag="p")
                    for kt in range(FT):
                        nc.tensor.matmul(
                            ps,
                            wk_t[kt][:, ft * 128 : (ft + 1) * 128],
                            xt_t[kt][:, nt * 512 : (nt + 1) * 512],
                            start=(kt == 0),
                            stop=(kt == FT - 1),
                        )
                    nc.vector.scalar_tensor_tensor(
                        out=sb_K[:, ft, nt * 512 : (nt + 1) * 512],
                        in0=ps,
                        scalar=sb_bk[:, ft : ft + 1],
                        in1=mask_bc[:, nt * 512 : (nt + 1) * 512],
                        op0=ALU.add,
                        op1=ALU.mult,
                    )
                # Q^T[ft] for the core's own block
                ps = ps_qkv.tile([128, 512], f32, tag="p")
                for kt in range(FT):
                    nc.tensor.matmul(
                        ps,
                        wq_t[kt][:, ft * 128 : (ft + 1) * 128],
                        xtb_t[kt],
                        start=(kt == 0),
                        stop=(kt == FT - 1),
                    )
                nc.vector.tensor_scalar_add(sb_Q[:, ft, :], ps, sb_bq[:, ft : ft + 1])

                if ft == 0:
                    # V tok-major over the whole batch, laid out [tok, head, dk+1].
                    # Must be complete before the first z matmul below.
                    for nh in range(2):
                        for tt2 in range(ST):
                            psv = ps_qkv.tile([128, 512], f32, tag="p")
                            for kt in range(FT):
                                nc.tensor.matmul(
                                    psv[:, 0:384],
                                    xt_t[kt][:, tt2 * 128 : (tt2 + 1) * 128],
                                    wv_t[kt][:, nh * 384 : (nh + 1) * 384],
                                    start=(kt == 0),
                                    stop=(kt == FT - 1),
                                )
                            nc.vector.scalar_tensor_tensor(
                                out=sb_V[:, tt2, nh * 6 : (nh + 1) * 6, 0:DK],
                                in0=psv[:, 0:384].rearrange("p (h d) -> p h d", d=DK),
                                scalar=1.0,
                                in1=bv_bc[:, nh * 384 : (nh + 1) * 384].rearrange(
                                    "p (h d) -> p h d", d=DK
                                ),
                                op0=ALU.mult,
                                op1=ALU.add,
                            )

                # scores + exp + z for the two heads of this feature tile
                for half in (0, 1):
                    h = 2 * ft + half
                    ho = half * 64
                    zp = ps_z.tile([128, BLK], f32, tag="z")
                    for b8 in range(ST // 2):
                        pssc = ps_sc.tile([128, 2, 512], f32, tag="sc")
                        for j in (0, 1):
                            kt2 = b8 * 2 + j
                            nc.tensor.matmul(
                                pssc[:, j, :],
                                sb_K[ho : ho + 64, ft, kt2 * 128 : (kt2 + 1) * 128],
                                sb_Q[ho : ho + 64, ft, :],
                                start=True,
                                stop=True,
                            )
                        et = ets_p.tile([128, 2, BLK], bf16, tag="exp")
                        nc.scalar.activation(
                            et[:].rearrange("p a b -> p (a b)"),
                            pssc[:].rearrange("p a b -> p (a b)"),
                            AF.Exp,
                            scale=ISCALE,
                        )
                        for j in (0, 1):
                            kt2 = b8 * 2 + j
                            nc.tensor.matmul(
                                zp[0 : DK + 1, :],
                                sb_V[:, kt2, h, :],
                                et[:, j, :],
                                start=(kt2 == 0),
                                stop=(kt2 == ST - 1),
                            )
                        if b8 == 2:
                            # previous head's normalization tail goes here so
                            # the PE never waits on the DVE reciprocal (the
                            # rank-1 matmul has a WAR on the denominator row
                            # the reciprocal reads; ~3us of scores gives the
                            # reciprocal time to drain)
                            flush_tail()
                    rsum = attsm.tile([1, BLK], f32, tag="rsum")
                    nc.vector.reciprocal(rsum, zp[DK : DK + 1, :])
                    pending.append((zp, rsum, ho, ft))

            flush_tail()

            if TAPS:
                nc.sync.dma_start(out=d_tap_zT[:], in_=sb_zT)

            ps_z_cm.__exit__(None, None, None)
            ps_sc_cm.__exit__(None, None, None)
            ps_qkv_cm.__exit__(None, None, None)
            attsm_cm.__exit__(None, None, None)
            ets_cm.__exit__(None, None, None)
            wqkv_cm.__exit__(None, None, None)
            xt_cm.__exit__(None, None, None)

            # ============ O proj + LN1 (+residual) + l1 transpose ============
            def layer_norm_to(out_ap, x_ap, g_bc_t, resid_ap, pool):
                s = pool.tile([128, 1], f32, tag="ln_s")
                nc.vector.tensor_reduce(s, x_ap, axis=AX.X, op=ALU.add)
                mean = pool.tile([128, 1], f32, tag="ln_m")
                nc.vector.tensor_scalar_mul(mean, s, 1.0 / DIM)
                xc = pool.tile([128, DIM], f32, tag="ln_xc")
                nc.vector.tensor_scalar(xc, x_ap, mean, None, op0=ALU.subtract)
                junk = pool.tile([128, DIM], bf16, tag="ln_j")
                var = pool.tile([128, 1], f32, tag="ln_v")
                nc.scalar.activation(junk, xc, AF.Square, accum_out=var)
                sd = pool.tile([128, 1], f32, tag="ln_sd")
                nc.scalar.activation(sd, var, AF.Sqrt, bias=eps_t[:], scale=1.0 / DIM)
                rstd = pool.tile([128, 1], f32, tag="ln_r")
                nc.vector.reciprocal(rstd, sd)
                tg = pool.tile([128, DIM], f32, tag="ln_tg")
                nc.vector.scalar_tensor_tensor(
                    out=tg, in0=xc, scalar=rstd, in1=g_bc_t,
                    op0=ALU.mult, op1=ALU.mult,
                )
                nc.vector.tensor_add(out_ap, tg, resid_ap)

            xb_r = d_xb[:].rearrange("(t p) d -> p t d", p=128)
            with (
                tc.tile_pool(name="ln1p", bufs=2) as ln1p,
                tc.tile_pool(name="ps_o", bufs=2, space="PSUM") as ps_o,
                tc.tile_pool(name="ps_t", bufs=2, space="PSUM") as ps_t,
            ):
                for tt in range(TT):
                    l1pre = ln1p.tile([128, DIM], f32, tag="l1pre")
                    for nh in range(2):
                        ps = ps_o.tile([128, 384], f32, tag="op")
                        for kt in range(FT):
                            nc.tensor.matmul(
                                ps,
                                sb_zT[:, kt, tt * 128 : (tt + 1) * 128],
                                wo_t[kt][:, nh * 384 : (nh + 1) * 384],
                                start=(kt == 0),
                                stop=(kt == FT - 1),
                            )
                        nc.vector.scalar_tensor_tensor(
                            out=l1pre[:, nh * 384 : (nh + 1) * 384],
                            in0=ps,
                            scalar=1.0,
                            in1=bo_bc[:, nh * 384 : (nh + 1) * 384],
                            op0=ALU.mult,
                            op1=ALU.add,
                        )
                    xbt = ln1p.tile([128, DIM], f32, tag="xbt")
                    nc.sync.dma_start(out=xbt, in_=xb_r[:, tt, :])
                    xb1 = ln1p.tile([128, DIM], f32, tag="xb1")
                    nc.vector.tensor_add(xb1, xbt, bb1_bc)
                    layer_norm_to(sb_l1[:, tt, :], l1pre[:], g1_bc, xb1, ln1p)
                    # transpose l1[tt] right away so FFN1 can start early
                    for ft in range(FT):
                        pst = ps_t.tile([128, 128], f32, tag="tp")
                        nc.tensor.transpose(
                            pst, sb_l1[:, tt, ft * 128 : (ft + 1) * 128], ident[:]
                        )
                        nc.vector.tensor_copy(
                            sb_l1T[:, ft, tt * 128 : (tt + 1) * 128], pst
                        )

            if TAPS:
                nc.sync.dma_start(out=d_tap_l1[:], in_=sb_l1)

            wo_cm.__exit__(None, None, None)
            attn_cm.__exit__(None, None, None)

            # ============ FFN1 -> hT, FFN2 streamed behind it ============
            w1_cm = tc.tile_pool(name="w1_p", bufs=1)
            w1_p = w1_cm.__enter__()
            w1_t = []
            for kt in range(FT):
                w1t = w1_p.tile([128, HID], bf16, tag=f"w1_{kt}")
                _eng[kt % 3].dma_start(
                    out=w1t, in_=d_w1[kt * 128 : (kt + 1) * 128, :]
                )
                w1_t.append(w1t)
            hT_cm = tc.tile_pool(name="hT_p", bufs=1)
            hT_p = hT_cm.__enter__()
            sb_hT = hT_p.tile([128, HT, BLK], bf16)  # relu(ffn1)^T, hid-major

            with (
                tc.tile_pool(name="w2s", bufs=1) as w2s_p,
                tc.tile_pool(name="ln2p", bufs=2) as ln2p,
                tc.tile_pool(name="f2pre_p", bufs=4) as f2pre_p,
                tc.tile_pool(name="outp", bufs=3) as outp,
                tc.tile_pool(name="ps_f1", bufs=2, space="PSUM") as ps_f1,
                tc.tile_pool(name="ps_f2", bufs=4, space="PSUM") as ps_f2,
            ):
                # prefetch all of w2 in 8 chunks (4 kt-groups x 2 halves)
                w2c = {}
                ci = 0
                for nh in range(2):
                    for kg in range(4):
                        w2t = w2s_p.tile([128, 6, 384], bf16, tag=f"w2c{nh}{kg}")
                        _eng[(ci + 2) % 3].dma_start(
                            out=w2t,
                            in_=d_w2[
                                kg * 768 : (kg + 1) * 768,
                                nh * 384 : (nh + 1) * 384,
                            ].rearrange("(t p) c -> p t c", p=128),
                        )
                        w2c[(nh, kg)] = w2t
                        ci += 1

                for ht2 in range(HT):
                    ps = ps_f1.tile([128, BLK], f32, tag="f1")
                    for kt in range(FT):
                        nc.tensor.matmul(
                            ps,
                            w1_t[kt][:, ht2 * 128 : (ht2 + 1) * 128],
                            sb_l1T[:, kt, :],
                            start=(kt == 0),
                            stop=(kt == FT - 1),
                        )
                    # relu(x + b1) on DVE: (x add b1) max 0
                    nc.vector.tensor_scalar(
                        sb_hT[:, ht2, :], ps, sb_b1[:, ht2 : ht2 + 1], 0.0,
                        op0=ALU.add, op1=ALU.max,
                    )

                f2pre = []
                for _tt in range(TT):
                    f2pre_t = f2pre_p.tile([128, DIM], f32, tag="f2pre")
                    f2pre.append(f2pre_t)
                for nh in range(2):
                    chains = []
                    for _tt in range(TT):
                        chain_t = ps_f2.tile([128, 384], f32, tag="f2")
                        chains.append(chain_t)
                    for kt in range(HT):
                        w2s = w2c[(nh, kt // 6)][:, kt % 6, :]
                        for tt in range(TT):
                            nc.tensor.matmul(
                                chains[tt],
                                sb_hT[:, kt, tt * 128 : (tt + 1) * 128],
                                w2s,
                                start=(kt == 0),
                                stop=(kt == HT - 1),
                            )
                    for tt in range(TT):
                        nc.vector.scalar_tensor_tensor(
                            out=f2pre[tt][:, nh * 384 : (nh + 1) * 384],
                            in0=chains[tt],
                            scalar=1.0,
                            in1=b2_bc[:, nh * 384 : (nh + 1) * 384],
                            op0=ALU.mult,
                            op1=ALU.add,
                        )

                out_r = d_out[:].rearrange("(t p) d -> p t d", p=128)
                for tt in range(TT):
                    l1b = ln2p.tile([128, DIM], f32, tag="l1b")
                    nc.vector.tensor_add(l1b, sb_l1[:, tt, :], bb2_bc)
                    o_sb = outp.tile([128, DIM], f32, tag="osb")
                    layer_norm_to(o_sb[:], f2pre[tt][:], g2_bc, l1b, ln2p)
                    nc.sync.dma_start(out=out_r[:, tt, :], in_=o_sb)

            hT_cm.__exit__(None, None, None)
            w1_cm.__exit__(None, None, None)

    return nc


def _get_nc(finalized=True):
    if "nc" not in _CACHE:
        _CACHE["nc"] = _build_program()
    nc = _CACHE["nc"]
    if finalized and not nc.is_finalized():
        nc.finalize()
    return nc


def make_in_maps(inputs: dict) -> list:
    x = np.asarray(inputs["x_n"], np.float32).reshape(B, S, DIM)
    mask = np.asarray(inputs["mask"]).reshape(B, S)
    w = {
        k: np.ascontiguousarray(np.asarray(inputs[k], np.float32).astype(BF16))
        for k in ("wq", "wk", "wv", "wo", "w1", "w2")
    }
    vecs = {
        "bq": inputs["bq"], "bk": inputs["bk"], "bv": inputs["bv"],
        "bo": inputs["bo"], "b1": inputs["b1"], "b2": inputs["b2"],
        "g1": inputs["ln1_g"], "bb1": inputs["ln1_b"],
        "g2": inputs["ln2_g"], "bb2": inputs["ln2_b"],
    }
    vecs = {k: np.ascontiguousarray(np.asarray(v, np.float32)) for k, v in vecs.items()}
    in_maps = []
    for c in range(N_CORES):
        b, blk = c // NBLK, c % NBLK
        xb_full = x[b]
        xT = np.ascontiguousarray(xb_full.T.astype(BF16))
        xblk = np.ascontiguousarray(xb_full[blk * BLK : (blk + 1) * BLK])
        xTb = np.ascontiguousarray(xblk.T.astype(BF16))
        msk = (mask[b] != 0).astype(np.float32)
        m = {"xT": xT, "xTb": xTb, "xb": xblk, "msk": msk}
        m.update(w)
        m.update(vecs)
        in_maps.append(m)
    return in_maps


def assemble(per_core_out: list) -> np.ndarray:
    blocks = [np.asarray(o, np.float32) for o in per_core_out]
    full = np.concatenate(blocks, axis=0).reshape(B, S, DIM)
    return full


def kernel(**inputs) -> np.ndarray:
    from concourse.bass_utils import run_bass_kernel_spmd

    nc = _get_nc()
    in_maps = make_in_maps(inputs)
    res = run_bass_kernel_spmd(nc, in_maps, list(range(N_CORES)))
    return assemble([r["out"] for r in res.results])
